# revision 38
# baseline (speedup 1.0000x reference)
"""MoE ConditionalLayer kernel for Trainium2 (8 NeuronCores, expert-parallel).

Problem: B=4096 rows, D=1024 features, C=8 conditions (experts).  Each row is
routed to one expert's 2-layer MLP (D->D relu D->D); reference semantics also
leak relu(b1[c]) @ W2[c] + b2[c] from every *other* expert into every row
(zero-masked rows still get biases).  That leak term is row-independent given
the routed expert, so it is applied on the host as a cheap per-expert
correction; the hardware kernel computes relu(x @ W1[c] + b1[c]) @ W2[c] for
the rows of expert c.

Sharding: expert-parallel - core c owns expert c's weights and the first 512
rows routed to it (gathered, transposed to feature-major, padded).  Rows
beyond 512 per expert (a handful, routing is near-balanced) are computed on
the host in fp32.

All operands ship as bf16 (halves HBM traffic vs fp32 and, measured, beats
the fp32r matmul path on accuracy since PE fp32r truncation is coarser than
bf16 input rounding).  PSUM accumulates fp32.  DRAM layouts are
partition-major so every DMA descriptor is one partition's full contiguous
payload (2-12 KB).

Schedule, per core with R=512 (measured ~44us on-core vs 74.8us baseline):
  - Inputs stream on TWO hw DGE queues concurrently: Sync carries the even
    A-stripes [W1k|xTk] and the per-bank output DMAs; the Activation queue
    carries odd A-stripes, bias, and W2.  Whole-stripe DMAs beat finer
    splits: each DMA completion carries ~1.1us of DGE latency, so more,
    smaller DMAs on the critical path lose.
  - PE: warm-up matmuls on garbage (10x512 + 4x128 cols) bridge from the
    hoisted start (~6.1us) to stripe-0 readiness (~11.4us).  This must be
    gapless: a >1us PE idle resets the HAM activity window and drops the
    clock to 1.2GHz for ~3.4us of real work (~+3us).  Emission is
    Block-less (saves the block-exit drain+barrier) and a post-build IR
    splice hoists A0, A1 and the warm-ups to the very front of the main
    block — ahead of the framework's register inits and init barrier,
    which none of them depend on (~0.9us earlier issue).
  - L1: single k-sweep over the 8 psum banks (k outer, m inner), gated
    per-stripe on the alternating queues.  The final sweep's stop-matmuls
    release banks one by one; DVE (h0,h2,h4,h6) and ACT (h1,h3,h5,h7)
    evacuate with fused bias+relu DURING the final sweep, in h-consumption
    order, so L2 chases with near-zero junction stall.  A dummy activation
    right after ACT's DMA issues forces the ACT-table load during idle
    time (otherwise it lands at the junction and costs ~1.3us).
  - L2: m outer, k inner; only bank 0 carries per-k waits (program order
    covers the rest).  Banks complete every 1.73us; DVE/ACT alternate
    psum->bf16 copies and Sync streams one output DMA per bank.  The last
    output bank runs as two sequential 256-col half-sweeps (half A in psum
    bank 7, half B in long-free bank 0 — a psum bank must never have two
    concurrent clients, which is also why DVE alone evacuates both halves),
    so half A's evac+DMA overlaps half B's matmuls and the tail after the
    final matmul is one 256-col evac + one small DMA.
"""

import sys

for _p in ("/opt/trn_rl_repo", "/root/.axon_site/_ro/trn_rl_repo"):
    if _p not in sys.path:
        sys.path.append(_p)

import numpy as np

B, D, C = 4096, 1024, 8
P = 128
KT = D // P        # 8 k-tiles (and 8 m-tiles)
R = 512            # device row capacity per expert (PSUM-exact)
SA = D + R         # A-stripe cols (bf16): W1 row | xT row

_NC_CACHE: dict = {}


def _build_nc():
    from contextlib import ExitStack

    import concourse.bass as bass
    from concourse import mybir

    f32 = mybir.dt.float32
    bf16 = mybir.dt.bfloat16
    Alu = mybir.AluOpType
    Act = mybir.ActivationFunctionType

    nc = bass.Bass(enable_partition_id=False)
    pkA = nc.declare_dram_parameter("pkA", [P, KT * SA], bf16, isOutput=False)
    pkB = nc.declare_dram_parameter("pkB", [P, KT * D], bf16, isOutput=False)
    pbias = nc.declare_dram_parameter("pbias", [P, KT], f32, isOutput=False)
    outO = nc.declare_dram_parameter("outO", [P, KT * R], bf16, isOutput=True)

    # L1 stripe gates: stripe k -> (which queue sem, threshold).  Sync queue
    # carries [A0, A2, A4, A6]; Activation queue [A1, A3, A5, A7, bias, B03, B47].
    thrS = {0: 16, 2: 32, 4: 48, 6: 64}
    thrV = {1: 16, 3: 32, 5: 48, 7: 64}
    thrBIAS = 80  # on qsemV
    # L1 evac ownership alternates by k parity so h0..h7 become ready in
    # consumption order with both engines pipelining: DVE h0,h2,h4,h6
    # (evacD 1-4); ACT h1,h3,h5,h7 (evacA 1-4)
    L1D, L1A = (0, 2, 4, 6), (1, 3, 5, 7)
    # bank m -> (sem, value) for its L1 evacuation (bank-free / h-ready)
    V1 = {0: (0, 1), 2: (0, 2), 4: (0, 3), 6: (0, 4),
          1: (1, 1), 3: (1, 2), 5: (1, 3), 7: (1, 4)}
    # L2 evac ownership: DVE m0,m2,m4,m6 (evacD 5-8); ACT m1,m3,m5 (evacA 5-7).
    # Bank 7 (the tail) runs as two sequential 256-col half-sweeps on the PE
    # (psem 16, 17); DVE alone evacuates each half (evacD 9, 10) so half A's
    # evac+DMA overlaps half B's matmuls and the final chain is halved.
    L2D, L2A = (0, 2, 4, 6), (1, 3, 5)
    V2 = {m: (0, 5 + i) for i, m in enumerate(L2D)}
    V2.update({m: (1, 5 + i) for i, m in enumerate(L2A)})

    with ExitStack() as ctx:
        wa = ctx.enter_context(nc.sbuf_tensor("wa", [P, KT * SA], bf16))
        wb = ctx.enter_context(nc.sbuf_tensor("wb", [P, KT * D], bf16))
        hb = ctx.enter_context(nc.sbuf_tensor("hb", [P, KT * R], bf16))
        ob = ctx.enter_context(nc.sbuf_tensor("ob", [P, KT * R], bf16))
        bias = ctx.enter_context(nc.sbuf_tensor("bias", [P, KT], f32))
        scratch = ctx.enter_context(nc.sbuf_tensor("scratch", [P, 1], bf16))
        ps = [ctx.enter_context(nc.psum_tensor(f"ps_{m}", [P, 512], f32)) for m in range(KT)]
        qsemS = ctx.enter_context(nc.semaphore("qsemS"))
        qsemV = ctx.enter_context(nc.semaphore("qsemV"))
        qsemB = ctx.enter_context(nc.semaphore("qsemB"))
        psem = ctx.enter_context(nc.semaphore("psem"))
        evacD = ctx.enter_context(nc.semaphore("evacD"))
        evacA = ctx.enter_context(nc.semaphore("evacA"))
        osem = ctx.enter_context(nc.semaphore("osem"))
        evacs = (evacD, evacA)
        sync, vector, scalar, tensor = nc.sync, nc.vector, nc.scalar, nc.tensor

        def w1(k, m):
            return wa[:, k * SA + m * P:k * SA + (m + 1) * P]

        def xa(k):
            return wa[:, k * SA + D:k * SA + D + R]

        def w2(k, m):
            return wb[:, k * D + m * P:k * D + (m + 1) * P]

        def hB(k):
            return hb[:, k * R:(k + 1) * R]

        def oB(m):
            return ob[:, m * R:(m + 1) * R]

        if True:
            # even A-stripes (whole)
            for k in (0, 2, 4, 6):
                sync.dma_start(
                    out=wa[:, k * SA:(k + 1) * SA],
                    in_=pkA[:, k * SA:(k + 1) * SA],
                    single_packet=(k == 0),
                ).then_inc(qsemS, 16)
            # per-bank output DMAs, chasing the L2 evacuations
            for m in range(KT - 1):
                e, v = V2[m]
                sync.wait_ge(evacs[e], v)
                sync.dma_start(
                    out=outO[:, m * R:(m + 1) * R], in_=oB(m),
                ).then_inc(osem, 16)
            sync.wait_ge(evacD, 9)
            sync.dma_start(
                out=outO[:, 7 * R:7 * R + 256], in_=ob[:, 7 * R:7 * R + 256],
            ).then_inc(osem, 16)
            sync.wait_ge(evacD, 10)
            sync.dma_start(
                out=outO[:, 7 * R + 256:8 * R], in_=ob[:, 7 * R + 256:8 * R],
            ).then_inc(osem, 16)
            sync.wait_ge(osem, 144)

        if True:
            # L1 evac: fused bias + relu (f32 psum -> bf16 h)
            vector.wait_ge(qsemV, thrBIAS)
            for m in L1D:
                vector.wait_ge(psem, m + 1)
                vector.tensor_scalar(
                    hB(m), ps[m][:],
                    bias[:, m:m + 1], 0.0, Alu.add, Alu.max,
                ).then_inc(evacD, 1)
            # L2 evac: copy (f32 psum -> bf16 out staging)
            for m in L2D:
                vector.wait_ge(psem, 8 + m + 1)
                vector.tensor_scalar_add(oB(m), ps[m][:], 0.0).then_inc(evacD, 1)
            vector.wait_ge(psem, 16)
            vector.tensor_scalar_add(
                ob[:, 7 * R:7 * R + 256], ps[7][:, 0:256], 0.0,
            ).then_inc(evacD, 1)
            vector.wait_ge(psem, 17)
            vector.tensor_scalar_add(
                ob[:, 7 * R + 256:8 * R], ps[0][:, 0:256], 0.0,
            ).then_inc(evacD, 1)

        if True:
            # odd A-stripes, bias, then W2 halves — on the Activation hw queue
            for k in (1, 3, 5, 7):
                scalar.dma_start(
                    out=wa[:, k * SA:(k + 1) * SA],
                    in_=pkA[:, k * SA:(k + 1) * SA],
                    single_packet=(k == 1),
                ).then_inc(qsemV, 16)
            scalar.dma_start(out=bias[:], in_=pbias[:]).then_inc(qsemV, 16)
            scalar.dma_start(out=wb[:, 0:4 * D], in_=pkB[:, 0:4 * D]).then_inc(qsemB, 16)
            scalar.dma_start(out=wb[:, 4 * D:8 * D], in_=pkB[:, 4 * D:8 * D]).then_inc(qsemB, 16)
            # dummy activation: forces the ACT-table load during idle time
            # (otherwise it lands right before the first L1 evacuation and
            # delays the whole ACT evac chain by ~1.3us)
            scalar.activation(scratch[:], scratch[:], Act.Relu)
            scalar.wait_ge(qsemV, thrBIAS)
            for m in L1A:
                scalar.wait_ge(psem, m + 1)
                scalar.activation(
                    hB(m), ps[m][:], Act.Relu, bias=bias[:, m:m + 1],
                ).then_inc(evacA, 1)
            for m in L2A:
                scalar.wait_ge(psem, 8 + m + 1)
                scalar.activation(oB(m), ps[m][:], Act.Copy).then_inc(evacA, 1)

        if True:
            # warm-up: keep PE busy through the HAM ramp while stripe 0
            # streams in (results discarded; sized to end at data-ready
            # ~4us after the init barrier — a gap here resets the HAM
            # activity window and costs ~3us of half-clock matmuls).
            for i in range(10):
                tensor.matmul(ps[6 + (i % 2)][:], ob[:, 0:P], ob[:, R:2 * R],
                              start=True, stop=True)
            for i in range(4):
                tensor.matmul(ps[6 + (i % 2)][:, 0:P], ob[:, 0:P], ob[:, R:R + P],
                              start=True, stop=True)
            # layer 1: single k-sweep over the 8 m-banks, stripes alternate
            # between the Sync and Vector dma queues
            for k in range(KT):
                if k in thrS:
                    tensor.wait_ge(qsemS, thrS[k])
                if k in thrV:
                    tensor.wait_ge(qsemV, thrV[k])
                for m in range(KT):
                    mm = tensor.matmul(
                        ps[m][:], w1(k, m), xa(k),
                        start=(k == 0), stop=(k == KT - 1),
                    )
                    if k == KT - 1:
                        mm.then_inc(psem, 1)
            # layer 2: m outer, k inner.  Bank m=0 chases the L1
            # evacuations per-k; banks 1-7 are covered by program order.
            for m in range(KT - 1):
                for k in range(KT):
                    if m == 0:
                        if k == 0:
                            tensor.wait_ge(qsemB, 16)
                        elif k == 4:
                            tensor.wait_ge(qsemB, 32)
                        e, v = V1[k]
                        tensor.wait_ge(evacs[e], v)
                    mm = tensor.matmul(
                        ps[m][:], w2(k, m), hB(k),
                        start=(k == 0), stop=(k == KT - 1),
                    )
                    if k == KT - 1:
                        mm.then_inc(psem, 1)
            # bank 7 of the output: two sequential 256-col half-sweeps so
            # the first half's evac+DMA overlaps the second half's matmuls.
            # Half A accumulates in psum bank 7, half B in bank 0 (free since
            # its L2 evacuation) so no psum bank ever has two clients at once.
            for half, (bank, poff) in enumerate(((7, 0), (0, 0))):
                c0 = half * 256
                if bank == 0:
                    # bank 0 must be evacuated (evacD 5) before reuse; in
                    # practice ~9us earlier, but make it formal
                    tensor.wait_ge(evacD, 5)
                for k in range(KT):
                    mm = tensor.matmul(
                        ps[bank][:, poff:poff + 256], w2(k, 7),
                        hb[:, k * R + c0:k * R + c0 + 256],
                        start=(k == 0), stop=(k == KT - 1),
                    )
                    if k == KT - 1:
                        mm.then_inc(psem, 1)

    # Hoist latency-critical openers above the framework's init barrier:
    # the first input DMA on each hw queue (A0 on SP, A1 on ACT) and the PE
    # warm-up matmuls depend only on the DGE rings (configured in the NEFF
    # preamble), not on the const-ap memsets that barrier guards, so they
    # can issue ~0.5us earlier — data lands sooner and the HAM clock ramp
    # starts sooner.
    ET = mybir.EngineType
    insts = nc.m.functions[0].blocks[0].instructions
    sp_dma = next(t for t in insts
                  if isinstance(t, mybir.InstDMACopy) and t.engine == ET.SP)
    act_dma = next(t for t in insts
                   if isinstance(t, mybir.InstDMACopy) and t.engine == ET.Activation)
    warms = [t for t in insts
             if isinstance(t, mybir.InstMatmult) and t.engine == ET.PE][:14]
    moved = [sp_dma, act_dma] + warms
    for t in moved:
        insts.remove(t)
    insts[1:1] = moved

    return nc


def kernel(x, cond_ids, W1, b1, W2, b2, _want_trace=False):
    import ml_dtypes

    from concourse.bass_utils import run_bass_kernel_spmd

    bf = ml_dtypes.bfloat16
    x = np.ascontiguousarray(np.asarray(x, dtype=np.float32))
    cid = np.asarray(cond_ids).astype(np.int64)
    W1 = np.asarray(W1, dtype=np.float32)
    b1 = np.asarray(b1, dtype=np.float32)
    W2 = np.asarray(W2, dtype=np.float32)
    b2 = np.asarray(b2, dtype=np.float32)

    if "nc" not in _NC_CACHE:
        _NC_CACHE["nc"] = _build_nc()
    nc = _NC_CACHE["nc"]

    counts = np.bincount(cid, minlength=C)
    order = np.argsort(cid, kind="stable")
    bounds = np.concatenate([[0], np.cumsum(counts)])

    W1b = W1.astype(bf)   # [C, D, D]
    W2b = W2.astype(bf)
    xb = x.astype(bf)

    in_maps = []
    dev_rows_all = []
    host_rows_all = []
    for c in range(C):
        rows = order[bounds[c]:bounds[c + 1]]
        dev_rows, host_rows = rows[:R], rows[R:]
        dev_rows_all.append(dev_rows)
        host_rows_all.append(host_rows)

        pkA = np.zeros((P, KT, SA), bf)
        pkA[:, :, :D] = W1b[c].reshape(KT, P, D).transpose(1, 0, 2)
        nr = len(dev_rows)
        if nr:
            pkA[:, :, D:D + nr] = xb[dev_rows].reshape(nr, KT, P).transpose(2, 1, 0)
        pkB = np.ascontiguousarray(
            W2b[c].reshape(KT, P, D).transpose(1, 0, 2)).reshape(P, KT * D)
        pbias = np.ascontiguousarray(b1[c].reshape(KT, P).T)
        in_maps.append({
            "pkA": pkA.reshape(P, KT * SA),
            "pkB": pkB,
            "pbias": pbias,
        })

    # Dry-run once to absorb first-execution-after-load cold-start effects
    # (cold DGE/ucode paths showed a rare partial-data race on the very first
    # execution of a freshly compiled NEFF); return the warm second run.
    run_bass_kernel_spmd(nc, in_maps, list(range(C)), trace=False)
    res = run_bass_kernel_spmd(nc, in_maps, list(range(C)), trace=_want_trace)

    out = np.empty((B, D), np.float32)
    for c in range(C):
        dev_rows, host_rows = dev_rows_all[c], host_rows_all[c]
        o = res.results[c]["outO"].astype(np.float32)  # [P, KT*R]
        nr = len(dev_rows)
        out[dev_rows] = o.reshape(P, KT, R).transpose(2, 1, 0)[:nr].reshape(nr, D)
        if len(host_rows):
            h = np.maximum(x[host_rows] @ W1[c] + b1[c], 0.0)
            out[host_rows] = h @ W2[c]

    # Reference leaks every expert's bias response through zero-masked rows:
    # out_true[b] = relu(x@W1[cb]+b1[cb])@W2[cb] + b2[cb] + sum_{c!=cb} z[c],
    # z[c] = relu(b1[c]) @ W2[c] + b2[c].  Kernel computed the first term
    # minus b2; add the rest here (exactly zero for zero biases).
    if b1.any() or b2.any():
        z = np.einsum("cd,cde->ce", np.maximum(b1, 0.0), W2) + b2
        corr = b2 + z.sum(axis=0)[None, :] - z
        out += corr[cid]

    if _want_trace:
        kernel._last_results = res
    return out


# revision 39
# speedup vs baseline: 1.0270x; 1.0270x over previous
"""MoE ConditionalLayer kernel for Trainium2 (8 NeuronCores, expert-parallel).

Problem: B=4096 rows, D=1024 features, C=8 conditions (experts).  Each row is
routed to one expert's 2-layer MLP (D->D relu D->D); reference semantics also
leak relu(b1[c]) @ W2[c] + b2[c] from every *other* expert into every row
(zero-masked rows still get biases).  That leak term is row-independent given
the routed expert, so it is applied on the host as a cheap per-expert
correction; the hardware kernel computes relu(x @ W1[c] + b1[c]) @ W2[c] for
the rows of expert c.

Sharding: expert-parallel - core c owns expert c's weights and the first 512
rows routed to it (gathered, transposed to feature-major, padded).  Rows
beyond 512 per expert (a handful, routing is near-balanced) are computed on
the host in fp32.

All operands ship as bf16 (halves HBM traffic vs fp32 and, measured, beats
the fp32r matmul path on accuracy since PE fp32r truncation is coarser than
bf16 input rounding).  PSUM accumulates fp32.  DRAM layouts are
partition-major so every DMA descriptor is one partition's full contiguous
payload (2-12 KB).

Schedule, per core with R=512 (measured ~44us on-core vs 74.8us baseline):
  - Inputs stream on TWO hw DGE queues concurrently: Sync carries the even
    A-stripes [W1k|xTk] and the per-bank output DMAs; the Activation queue
    carries odd A-stripes, bias, and W2.  Whole-stripe DMAs beat finer
    splits: each DMA completion carries ~1.1us of DGE latency, so more,
    smaller DMAs on the critical path lose.
  - PE: warm-up matmuls on garbage (10x512 + 4x128 cols) bridge from the
    hoisted start (~6.1us) to stripe-0 readiness (~11.4us).  This must be
    gapless: a >1us PE idle resets the HAM activity window and drops the
    clock to 1.2GHz for ~3.4us of real work (~+3us).  Emission is
    Block-less (saves the block-exit drain+barrier) and a post-build IR
    splice hoists A0, A1 and the warm-ups to the very front of the main
    block — ahead of the framework's register inits and init barrier,
    which none of them depend on (~0.9us earlier issue).
  - L1: single k-sweep over the 8 psum banks (k outer, m inner), gated
    per-stripe on the alternating queues.  The final sweep's stop-matmuls
    release banks one by one; DVE (h0,h2,h4,h6) and ACT (h1,h3,h5,h7)
    evacuate with fused bias+relu DURING the final sweep, in h-consumption
    order, so L2 chases with near-zero junction stall.  A dummy activation
    right after ACT's DMA issues forces the ACT-table load during idle
    time (otherwise it lands at the junction and costs ~1.3us).
  - L2: m outer, k inner; only bank 0 carries per-k waits (program order
    covers the rest).  Banks complete every 1.73us; DVE/ACT alternate
    psum->bf16 copies and Sync streams one output DMA per bank.  The last
    output bank runs as two sequential 256-col half-sweeps (half A in psum
    bank 7, half B in long-free bank 0 — a psum bank must never have two
    concurrent clients, which is also why DVE alone evacuates both halves),
    so half A's evac+DMA overlaps half B's matmuls and the tail after the
    final matmul is one 256-col evac + one small DMA.
"""

import sys

for _p in ("/opt/trn_rl_repo", "/root/.axon_site/_ro/trn_rl_repo"):
    if _p not in sys.path:
        sys.path.append(_p)

import numpy as np

B, D, C = 4096, 1024, 8
P = 128
KT = D // P        # 8 k-tiles (and 8 m-tiles)
R = 512            # device row capacity per expert (PSUM-exact)
SA = D + R         # A-stripe cols (bf16): W1 row | xT row

_NC_CACHE: dict = {}


def _build_nc():
    from contextlib import ExitStack

    import concourse.bass as bass
    from concourse import mybir

    f32 = mybir.dt.float32
    bf16 = mybir.dt.bfloat16
    Alu = mybir.AluOpType
    Act = mybir.ActivationFunctionType

    nc = bass.Bass(enable_partition_id=False)
    pkA = nc.declare_dram_parameter("pkA", [P, KT * SA], bf16, isOutput=False)
    pkB = nc.declare_dram_parameter("pkB", [P, KT * D], bf16, isOutput=False)
    pbias = nc.declare_dram_parameter("pbias", [P, KT], f32, isOutput=False)
    outO = nc.declare_dram_parameter("outO", [P, KT * R], bf16, isOutput=True)

    # L1 stripe gates: stripe k -> (which queue sem, threshold).  Sync queue
    # carries [A0, A2, A4, A6]; Activation queue [A1, A3, A5, A7, bias, B03, B47].
    thrS = {0: 16, 2: 32, 4: 48, 6: 64}
    thrV = {1: 16, 3: 32, 5: 48, 7: 64}
    thrBIAS = 80  # on qsemV
    # L1 evac ownership alternates by k parity so h0..h7 become ready in
    # consumption order with both engines pipelining: DVE h0,h2,h4,h6
    # (evacD 1-4); ACT h1,h3,h5,h7 (evacA 1-4)
    L1D, L1A = (0, 2, 4, 6), (1, 3, 5, 7)
    # bank m -> (sem, value) for its L1 evacuation (bank-free / h-ready)
    V1 = {0: (0, 1), 2: (0, 2), 4: (0, 3), 6: (0, 4),
          1: (1, 1), 3: (1, 2), 5: (1, 3), 7: (1, 4)}
    # L2 evac ownership: DVE m0,m2,m4,m6 (evacD 5-8); ACT m1,m3,m5 (evacA 5-7).
    # Bank 7 (the tail) runs as two sequential 256-col half-sweeps on the PE
    # (psem 16, 17); DVE alone evacuates each half (evacD 9, 10) so half A's
    # evac+DMA overlaps half B's matmuls and the final chain is halved.
    L2D, L2A = (0, 2, 4, 6), (1, 3, 5)
    V2 = {m: (0, 5 + i) for i, m in enumerate(L2D)}
    V2.update({m: (1, 5 + i) for i, m in enumerate(L2A)})

    with ExitStack() as ctx:
        wa = ctx.enter_context(nc.sbuf_tensor("wa", [P, KT * SA], bf16))
        wb = ctx.enter_context(nc.sbuf_tensor("wb", [P, KT * D], bf16))
        hb = ctx.enter_context(nc.sbuf_tensor("hb", [P, KT * R], bf16))
        ob = ctx.enter_context(nc.sbuf_tensor("ob", [P, KT * R], bf16))
        bias = ctx.enter_context(nc.sbuf_tensor("bias", [P, KT], f32))
        scratch = ctx.enter_context(nc.sbuf_tensor("scratch", [P, 1], bf16))
        ps = [ctx.enter_context(nc.psum_tensor(f"ps_{m}", [P, 512], f32)) for m in range(KT)]
        qsemS = ctx.enter_context(nc.semaphore("qsemS"))
        qsemV = ctx.enter_context(nc.semaphore("qsemV"))
        qsemB = ctx.enter_context(nc.semaphore("qsemB"))
        psem = ctx.enter_context(nc.semaphore("psem"))
        evacD = ctx.enter_context(nc.semaphore("evacD"))
        evacA = ctx.enter_context(nc.semaphore("evacA"))
        osem = ctx.enter_context(nc.semaphore("osem"))
        evacs = (evacD, evacA)
        sync, vector, scalar, tensor = nc.sync, nc.vector, nc.scalar, nc.tensor

        def w1(k, m):
            return wa[:, k * SA + m * P:k * SA + (m + 1) * P]

        def xa(k):
            return wa[:, k * SA + D:k * SA + D + R]

        def w2(k, m):
            return wb[:, k * D + m * P:k * D + (m + 1) * P]

        def hB(k):
            return hb[:, k * R:(k + 1) * R]

        def oB(m):
            return ob[:, m * R:(m + 1) * R]

        if True:
            # even A-stripes (whole)
            for k in (0, 2, 4, 6):
                sync.dma_start(
                    out=wa[:, k * SA:(k + 1) * SA],
                    in_=pkA[:, k * SA:(k + 1) * SA],
                ).then_inc(qsemS, 16)
            # per-bank output DMAs, chasing the L2 evacuations
            for m in range(KT - 1):
                e, v = V2[m]
                sync.wait_ge(evacs[e], v)
                sync.dma_start(
                    out=outO[:, m * R:(m + 1) * R], in_=oB(m),
                ).then_inc(osem, 16)
            sync.wait_ge(evacD, 9)
            sync.dma_start(
                out=outO[:, 7 * R:7 * R + 256], in_=ob[:, 7 * R:7 * R + 256],
            ).then_inc(osem, 16)
            sync.wait_ge(evacD, 10)
            sync.dma_start(
                out=outO[:, 7 * R + 256:8 * R], in_=ob[:, 7 * R + 256:8 * R],
            ).then_inc(osem, 16)
            sync.wait_ge(osem, 144)

        if True:
            # L1 evac: fused bias + relu (f32 psum -> bf16 h)
            vector.wait_ge(qsemV, thrBIAS)
            for m in L1D:
                vector.wait_ge(psem, m + 1)
                vector.tensor_scalar(
                    hB(m), ps[m][:],
                    bias[:, m:m + 1], 0.0, Alu.add, Alu.max,
                ).then_inc(evacD, 1)
            # L2 evac: copy (f32 psum -> bf16 out staging)
            for m in L2D:
                vector.wait_ge(psem, 8 + m + 1)
                vector.tensor_scalar_add(oB(m), ps[m][:], 0.0).then_inc(evacD, 1)
            vector.wait_ge(psem, 16)
            vector.tensor_scalar_add(
                ob[:, 7 * R:7 * R + 256], ps[7][:, 0:256], 0.0,
            ).then_inc(evacD, 1)
            vector.wait_ge(psem, 17)
            vector.tensor_scalar_add(
                ob[:, 7 * R + 256:8 * R], ps[0][:, 0:256], 0.0,
            ).then_inc(evacD, 1)

        if True:
            # odd A-stripes, bias, then W2 halves — on the Activation hw queue
            for k in (1, 3, 5, 7):
                scalar.dma_start(
                    out=wa[:, k * SA:(k + 1) * SA],
                    in_=pkA[:, k * SA:(k + 1) * SA],
                ).then_inc(qsemV, 16)
            scalar.dma_start(out=bias[:], in_=pbias[:]).then_inc(qsemV, 16)
            scalar.dma_start(out=wb[:, 0:4 * D], in_=pkB[:, 0:4 * D]).then_inc(qsemB, 16)
            scalar.dma_start(out=wb[:, 4 * D:8 * D], in_=pkB[:, 4 * D:8 * D]).then_inc(qsemB, 16)
            # dummy activation: forces the ACT-table load during idle time
            # (otherwise it lands right before the first L1 evacuation and
            # delays the whole ACT evac chain by ~1.3us)
            scalar.activation(scratch[:], scratch[:], Act.Relu)
            scalar.wait_ge(qsemV, thrBIAS)
            for m in L1A:
                scalar.wait_ge(psem, m + 1)
                scalar.activation(
                    hB(m), ps[m][:], Act.Relu, bias=bias[:, m:m + 1],
                ).then_inc(evacA, 1)
            for m in L2A:
                scalar.wait_ge(psem, 8 + m + 1)
                scalar.activation(oB(m), ps[m][:], Act.Copy).then_inc(evacA, 1)

        if True:
            # warm-up: keep PE busy through the HAM ramp while stripe 0
            # streams in (results discarded; sized to end at data-ready
            # ~4us after the init barrier — a gap here resets the HAM
            # activity window and costs ~3us of half-clock matmuls).
            for i in range(10):
                tensor.matmul(ps[6 + (i % 2)][:], ob[:, 0:P], ob[:, R:2 * R],
                              start=True, stop=True)
            for i in range(4):
                tensor.matmul(ps[6 + (i % 2)][:, 0:P], ob[:, 0:P], ob[:, R:R + P],
                              start=True, stop=True)
            # layer 1: single k-sweep over the 8 m-banks, stripes alternate
            # between the Sync and Vector dma queues
            for k in range(KT):
                if k in thrS:
                    tensor.wait_ge(qsemS, thrS[k])
                if k in thrV:
                    tensor.wait_ge(qsemV, thrV[k])
                for m in range(KT):
                    mm = tensor.matmul(
                        ps[m][:], w1(k, m), xa(k),
                        start=(k == 0), stop=(k == KT - 1),
                    )
                    if k == KT - 1:
                        mm.then_inc(psem, 1)
            # layer 2: m outer, k inner.  Bank m=0 chases the L1
            # evacuations per-k; banks 1-7 are covered by program order.
            for m in range(KT - 1):
                for k in range(KT):
                    if m == 0:
                        if k == 0:
                            tensor.wait_ge(qsemB, 16)
                        elif k == 4:
                            tensor.wait_ge(qsemB, 32)
                        e, v = V1[k]
                        tensor.wait_ge(evacs[e], v)
                    mm = tensor.matmul(
                        ps[m][:], w2(k, m), hB(k),
                        start=(k == 0), stop=(k == KT - 1),
                    )
                    if k == KT - 1:
                        mm.then_inc(psem, 1)
            # bank 7 of the output: two sequential 256-col half-sweeps so
            # the first half's evac+DMA overlaps the second half's matmuls.
            # Half A accumulates in psum bank 7, half B in bank 0 (free since
            # its L2 evacuation) so no psum bank ever has two clients at once.
            for half, (bank, poff) in enumerate(((7, 0), (0, 0))):
                c0 = half * 256
                if bank == 0:
                    # bank 0 must be evacuated (evacD 5) before reuse; in
                    # practice ~9us earlier, but make it formal
                    tensor.wait_ge(evacD, 5)
                for k in range(KT):
                    mm = tensor.matmul(
                        ps[bank][:, poff:poff + 256], w2(k, 7),
                        hb[:, k * R + c0:k * R + c0 + 256],
                        start=(k == 0), stop=(k == KT - 1),
                    )
                    if k == KT - 1:
                        mm.then_inc(psem, 1)

    # Hoist latency-critical openers above the framework's init barrier:
    # the first input DMA on each hw queue (A0 on SP, A1 on ACT) and the PE
    # warm-up matmuls depend only on the DGE rings (configured in the NEFF
    # preamble), not on the const-ap memsets that barrier guards, so they
    # can issue ~0.5us earlier — data lands sooner and the HAM clock ramp
    # starts sooner.
    ET = mybir.EngineType
    insts = nc.m.functions[0].blocks[0].instructions
    sp_dma = next(t for t in insts
                  if isinstance(t, mybir.InstDMACopy) and t.engine == ET.SP)
    act_dma = next(t for t in insts
                   if isinstance(t, mybir.InstDMACopy) and t.engine == ET.Activation)
    warms = [t for t in insts
             if isinstance(t, mybir.InstMatmult) and t.engine == ET.PE][:14]
    moved = [sp_dma, act_dma] + warms
    for t in moved:
        insts.remove(t)
    insts[1:1] = moved

    return nc


def kernel(x, cond_ids, W1, b1, W2, b2, _want_trace=False):
    import ml_dtypes

    from concourse.bass_utils import run_bass_kernel_spmd

    bf = ml_dtypes.bfloat16
    x = np.ascontiguousarray(np.asarray(x, dtype=np.float32))
    cid = np.asarray(cond_ids).astype(np.int64)
    W1 = np.asarray(W1, dtype=np.float32)
    b1 = np.asarray(b1, dtype=np.float32)
    W2 = np.asarray(W2, dtype=np.float32)
    b2 = np.asarray(b2, dtype=np.float32)

    if "nc" not in _NC_CACHE:
        _NC_CACHE["nc"] = _build_nc()
    nc = _NC_CACHE["nc"]

    counts = np.bincount(cid, minlength=C)
    order = np.argsort(cid, kind="stable")
    bounds = np.concatenate([[0], np.cumsum(counts)])

    W1b = W1.astype(bf)   # [C, D, D]
    W2b = W2.astype(bf)
    xb = x.astype(bf)

    in_maps = []
    dev_rows_all = []
    host_rows_all = []
    for c in range(C):
        rows = order[bounds[c]:bounds[c + 1]]
        dev_rows, host_rows = rows[:R], rows[R:]
        dev_rows_all.append(dev_rows)
        host_rows_all.append(host_rows)

        pkA = np.zeros((P, KT, SA), bf)
        pkA[:, :, :D] = W1b[c].reshape(KT, P, D).transpose(1, 0, 2)
        nr = len(dev_rows)
        if nr:
            pkA[:, :, D:D + nr] = xb[dev_rows].reshape(nr, KT, P).transpose(2, 1, 0)
        pkB = np.ascontiguousarray(
            W2b[c].reshape(KT, P, D).transpose(1, 0, 2)).reshape(P, KT * D)
        pbias = np.ascontiguousarray(b1[c].reshape(KT, P).T)
        in_maps.append({
            "pkA": pkA.reshape(P, KT * SA),
            "pkB": pkB,
            "pbias": pbias,
        })

    # Dry-run once to absorb first-execution-after-load cold-start effects
    # (cold DGE/ucode paths showed a rare partial-data race on the very first
    # execution of a freshly compiled NEFF); return the warm second run.
    run_bass_kernel_spmd(nc, in_maps, list(range(C)), trace=False)
    res = run_bass_kernel_spmd(nc, in_maps, list(range(C)), trace=_want_trace)

    out = np.empty((B, D), np.float32)
    for c in range(C):
        dev_rows, host_rows = dev_rows_all[c], host_rows_all[c]
        o = res.results[c]["outO"].astype(np.float32)  # [P, KT*R]
        nr = len(dev_rows)
        out[dev_rows] = o.reshape(P, KT, R).transpose(2, 1, 0)[:nr].reshape(nr, D)
        if len(host_rows):
            h = np.maximum(x[host_rows] @ W1[c] + b1[c], 0.0)
            out[host_rows] = h @ W2[c]

    # Reference leaks every expert's bias response through zero-masked rows:
    # out_true[b] = relu(x@W1[cb]+b1[cb])@W2[cb] + b2[cb] + sum_{c!=cb} z[c],
    # z[c] = relu(b1[c]) @ W2[c] + b2[c].  Kernel computed the first term
    # minus b2; add the rest here (exactly zero for zero biases).
    if b1.any() or b2.any():
        z = np.einsum("cd,cde->ce", np.maximum(b1, 0.0), W2) + b2
        corr = b2 + z.sum(axis=0)[None, :] - z
        out += corr[cid]

    if _want_trace:
        kernel._last_results = res
    return out
